# revision 15
# baseline (speedup 1.0000x reference)
"""GAT (2-layer multi-head graph attention) on 8 Trainium2 NeuronCores.

Sharding: nodes (rows of adj / attention) are sharded across the 8 cores;
each core computes h = x@W replicated, its 512-row block of
e/softmax/aggregation for both GAT layers, with an AllGather of the layer-1
output (xcat) between layers.

Layout trick: attention probabilities are computed TRANSPOSED (eT[j, i]) so
softmax-normalizer and aggregation both run on the tensor engine:
  aggT[o, i] = sum_j hplus[j, o] * P[j, i]  with hplus = [h | 1] so the last
row of the accumulator is the softmax denominator Z.  exp/leaky run on the
scalar engine (Prelu alpha=0.2 + Exp share one ACT table set), masking is a
single DVE scalar_tensor_tensor using (adj-1)*100 added before the leaky
(masked entries land at ~exp(-16) -> 0).
"""
import os
import sys

for _p in ("/opt/trn_rl_repo", "/root/.axon_site/_ro/trn_rl_repo"):
    if os.path.isdir(_p) and _p not in sys.path:
        sys.path.insert(0, _p)

import numpy as np
import ml_dtypes

import concourse.bacc as bacc
import concourse.mybir as mybir
import concourse.tile as tile
from concourse import bass_utils

F32 = mybir.dt.float32
F32R = mybir.dt.float32r
BF16 = mybir.dt.bfloat16
AF = mybir.ActivationFunctionType
ALU = mybir.AluOpType

N, NFEAT, NHID, NCLASS, NHEADS = 4096, 512, 64, 128, 8
NCORES = 8
R = N // NCORES          # 512 rows per core
FC = NFEAT // 128        # 4 feature chunks
JC = N // 128            # 32 j-chunks
BIG = 100.0
ALPHA = 0.2

_CACHE = {}


def _build_nc():
    nc = bacc.Bacc("TRN2", target_bir_lowering=False, debug=False,
                   num_devices=NCORES)

    xT_d = nc.dram_tensor("xT", [NFEAT, N], F32R, kind="ExternalInput")
    xTb_d = nc.dram_tensor("xTblk", [NFEAT, R], F32R, kind="ExternalInput")
    Wcat_d = nc.dram_tensor("Wcat", [NFEAT, 512], F32R, kind="ExternalInput")
    WcatT_d = nc.dram_tensor("WcatT", [512, NFEAT], F32R, kind="ExternalInput")
    A12_d = nc.dram_tensor("A12", [512, 16], F32R, kind="ExternalInput")
    Wout_d = nc.dram_tensor("Wout", [512, NCLASS], F32R, kind="ExternalInput")
    WoutT_d = nc.dram_tensor("WoutT", [NCLASS, 512], F32R, kind="ExternalInput")
    AO_d = nc.dram_tensor("AO", [NCLASS, 2], F32R, kind="ExternalInput")
    adj_d = nc.dram_tensor("adjm1T", [N, R], BF16, kind="ExternalInput")
    id_d = nc.dram_tensor("ident", [128, 128], F32, kind="ExternalInput")
    out_d = nc.dram_tensor("out", [R, NCLASS], F32, kind="ExternalOutput")

    with tile.TileContext(nc, num_cores=NCORES) as tc:
        with (
            tc.tile_pool(name="persist", bufs=1) as Pp,
            tc.tile_pool(name="dram", bufs=1, space="DRAM") as Pd,
            tc.tile_pool(name="psA", bufs=2, space="PSUM") as PsA,
            tc.tile_pool(name="psS", bufs=2, space="PSUM") as PsS,
            tc.tile_pool(name="pagg", bufs=1, space="PSUM") as Pagg,
        ):
            # ---- persistent constants / small state ----
            alpha = Pp.tile([128, 1], F32, name="alpha")
            nc.vector.memset(alpha[:], ALPHA)
            onescol = Pp.tile([128, 1], F32R, name="onescol")
            nc.vector.memset(onescol[:].bitcast(F32), 1.0)
            sfjT = Pp.tile([128, JC, 8], F32, name="sfjT")
            sxcb = Pp.tile([128, FC, R], F32R, name="sxcb")  # own xcatT block
            sw12 = Pp.tile([128, FC, 16], F32, name="sw12")
            sWout = Pp.tile([128, FC, NCLASS], F32R, name="sWout")
            for fc in range(FC):
                nc.sync.dma_start(
                    sWout[:, fc, :], Wout_d.ap()[fc * 128:(fc + 1) * 128, :])
            sWoutT = Pp.tile([128, 512], F32R, name="sWoutT")
            nc.sync.dma_start(sWoutT[:], WoutT_d.ap())
            sAO = Pp.tile([128, 2], F32R, name="sAO")
            nc.sync.dma_start(sAO[:], AO_d.ap())
            sw2 = Pp.tile([128, FC, 2], F32R, name="sw2")
            for fc in range(FC):
                pw2 = PsS.tile([128, 2], F32, tag="ps_s", bufs=2)
                nc.tensor.matmul(
                    pw2[:], sWoutT[:, fc * 128:(fc + 1) * 128], sAO[:],
                    start=True, stop=True)
                nc.vector.tensor_copy(sw2[:, fc, :], pw2[:])
            fibs = [Pp.tile([128, R], F32, name=f"fib{h}") for h in range(NHEADS)]

            with tc.tile_pool(name="hplusp", bufs=1) as Ph:
                shplus = Ph.tile([128, JC, NHEADS, NHID + 1], F32R, name="shplus")
                nc.vector.memset(shplus[:, :, :, NHID].bitcast(F32), 1.0)

                # ================= stage 1: weights / fifj =================
                with tc.tile_pool(name="stage1", bufs=1) as P1:
                    sxT = P1.tile([128, FC, N], F32R, name="sxT")
                    for fc in range(FC):
                        nc.sync.dma_start(
                            sxT[:, fc, :], xT_d.ap()[fc * 128:(fc + 1) * 128, :])
                    sWcat = P1.tile([128, FC, 512], F32R, name="sWcat")
                    for fc in range(FC):
                        nc.sync.dma_start(
                            sWcat[:, fc, :], Wcat_d.ap()[fc * 128:(fc + 1) * 128, :])
                    sfown = P1.tile([16, R], F32, name="sfown")

                    with tc.tile_pool(name="stage1a", bufs=1) as P1a:
                        sA12 = P1a.tile([128, 4, 16], F32, name="sA12")
                        for hoc in range(4):
                            nc.sync.dma_start(
                                sA12[:, hoc, :],
                                A12_d.ap()[hoc * 128:(hoc + 1) * 128, :].bitcast(F32))
                        sxTb = P1a.tile([128, FC, R], F32, name="sxTb")
                        for fc in range(FC):
                            nc.sync.dma_start(
                                sxTb[:, fc, :],
                                xTb_d.ap()[fc * 128:(fc + 1) * 128, :].bitcast(F32))

                        # w12[f, k] = sum_ho WcatT[ho, f] * A12[ho, k]
                        # stream 128x128 WcatT tiles per (fc, hoc)
                        for fc in range(FC):
                            pw = PsS.tile([128, 16], F32, tag="ps_s", bufs=2)
                            for hoc in range(4):
                                swcT = P1a.tile([128, 128], F32, tag="swcT",
                                                bufs=2)
                                nc.sync.dma_start(
                                    swcT[:],
                                    WcatT_d.ap()[hoc * 128:(hoc + 1) * 128,
                                                 fc * 128:(fc + 1) * 128]
                                    .bitcast(F32))
                                nc.tensor.matmul(
                                    pw[:], swcT[:], sA12[:, hoc, :],
                                    start=(hoc == 0), stop=(hoc == 3))
                            nc.vector.tensor_copy(sw12[:, fc, :], pw[:])

                        # fj columns directly: [j-chunk 128, 8] per jc (fp32)
                        for jc in range(JC):
                            pfj = PsS.tile([128, 8], F32, tag="ps_s", bufs=2)
                            for fc in range(FC):
                                xTfj = P1a.tile([128, 128], F32, tag="xTfj",
                                                bufs=3)
                                nc.sync.dma_start(
                                    xTfj[:],
                                    xT_d.ap()[fc * 128:(fc + 1) * 128,
                                              jc * 128:(jc + 1) * 128]
                                    .bitcast(F32))
                                nc.tensor.matmul(
                                    pfj[:], xTfj[:], sw12[:, fc, 0:8],
                                    start=(fc == 0), stop=(fc == 3))
                            nc.vector.tensor_copy(sfjT[:, jc, :], pfj[:])
                        # own-block fifj (for fi of this core's rows)
                        pfo = PsS.tile([16, 512], F32, tag="ps_s", bufs=2)
                        for fc in range(FC):
                            nc.tensor.matmul(
                                pfo[:], sw12[:, fc, :], sxTb[:, fc, :],
                                start=(fc == 0), stop=(fc == 3))
                        nc.vector.tensor_copy(sfown[:], pfo[:])

                    # fib[h] = broadcast of own-block fi row (partition 0 src)
                    for hd in range(NHEADS):
                        fot = Pp.tile([1, R], F32, tag="fot", bufs=2)
                        nc.sync.dma_start(fot[:], sfown[8 + hd:9 + hd, :])
                        nc.gpsimd.partition_broadcast(fibs[hd][:], fot[:])

                    # ============ stage A: hcat natural -> hplus ============
                    for ncx in range(JC):
                        pA = PsA.tile([128, 512], F32, tag="ps_a", bufs=2)
                        for fc in range(FC):
                            nc.tensor.matmul(
                                pA[:], sxT[:, fc, ncx * 128:(ncx + 1) * 128],
                                sWcat[:, fc, :],
                                start=(fc == 0), stop=(fc == 3))
                        nc.scalar.copy(
                            shplus[:, ncx, :, 0:NHID],
                            pA[:].rearrange("p (hd o) -> p hd o", o=NHID))

                # ================= layer-1 attention sweeps =================
                with tc.tile_pool(name="chunkL1", bufs=1) as Pc:
                    paggs = {}
                    for sweep in range(2):
                        heads = list(range(sweep * 4, sweep * 4 + 4))
                        for jc in range(JC):
                            mask = Pc.tile([128, 512], BF16, tag="mask", bufs=3)
                            nc.sync.dma_start(
                                mask[:], adj_d.ap()[jc * 128:(jc + 1) * 128, :])
                            raw4 = Pc.tile([128, 2048], F32, tag="raw4", bufs=2)
                            em4 = Pc.tile([128, 2048], F32, tag="em4", bufs=2)
                            P4 = Pc.tile([128, 2048], F32R, tag="p4", bufs=2)
                            for q, hd in enumerate(heads):
                                sl = slice(q * 512, (q + 1) * 512)
                                gidx = (sweep * JC + jc) * 4 + q
                                nc.vector.scalar_tensor_tensor(
                                    raw4[:, sl], mask[:], BIG, fibs[hd][:],
                                    op0=ALU.mult, op1=ALU.add)
                                if (gidx * 3) % 8 < 3:
                                    u = Pc.tile([128, 512], F32, tag="ulk",
                                                bufs=3)
                                    nc.vector.tensor_scalar_add(
                                        u[:], raw4[:, sl],
                                        sfjT[:, jc, hd:hd + 1])
                                    nc.vector.scalar_tensor_tensor(
                                        em4[:, sl], u[:], ALPHA, u[:],
                                        op0=ALU.mult, op1=ALU.max)
                                else:
                                    nc.scalar.activation(
                                        em4[:, sl], raw4[:, sl], AF.Prelu,
                                        bias=sfjT[:, jc, hd:hd + 1],
                                        alpha=alpha[:])
                            nc.scalar.activation(P4[:], em4[:], AF.Exp)
                            for q, hd in enumerate(heads):
                                if jc == 0:
                                    paggs[hd] = Pagg.tile(
                                        [NHID + 1, 512], F32, tag=f"agg{q}",
                                        bufs=1, name=f"agg_s{sweep}_{q}")
                                nc.tensor.matmul(
                                    paggs[hd][:], shplus[:, jc, hd, :],
                                    P4[:, q * 512:(q + 1) * 512],
                                    start=(jc == 0), stop=(jc == JC - 1))

                        # normalize this sweep's heads into the xcatT block
                        zsw = Pc.tile([4, R], F32, tag="zsw", bufs=2)
                        for q, hd in enumerate(heads):
                            zst = Pc.tile([NHID + 1, R], F32, tag="zst", bufs=2)
                            nc.vector.tensor_copy(
                                zst[NHID:NHID + 1, :], paggs[hd][NHID:NHID + 1, :])
                            nc.sync.dma_start(
                                zsw[q:q + 1, :], zst[NHID:NHID + 1, :])
                        rzw = Pc.tile([4, R], F32, tag="rzw", bufs=2)
                        nc.vector.reciprocal(rzw[:], zsw[:])
                        for q, hd in enumerate(heads):
                            rzt = Pc.tile([1, R], F32, tag="rzt", bufs=2)
                            nc.sync.dma_start(rzt[:], rzw[q:q + 1, :])
                            zb = Pc.tile([64, R], F32, tag="zb", bufs=2)
                            nc.gpsimd.partition_broadcast(zb[:], rzt[:])
                            xcn = Pc.tile([64, R], F32R, tag="xcn", bufs=2)
                            nc.vector.tensor_mul(
                                xcn[:], paggs[hd][0:NHID, :], zb[:])
                            nc.sync.dma_start(
                                sxcb[64 * (hd % 2):64 * (hd % 2) + 64,
                                     hd // 2, :], xcn[:])

            # ===== layer-2 projections on the OWN block, then small gather =====
            # h2_block[n, c] = sum_f xcat_blk[n, f] Wout[f, c]   (own 512 nodes)
            # fifj2_block = w2.T @ xcat_blkT  -> fi2 (row 0, local), fj2 (row 1)
            dblk2 = Pd.tile([R, NCLASS], F32, name="dblk2")
            dgath2 = Pd.tile([N, NCLASS], F32, name="dgath2",
                             addr_space="Shared")
            dblk2b = Pd.tile([1, R], F32, name="dblk2b")
            dgath2b = Pd.tile([8, R], F32, name="dgath2b",
                              addr_space="Shared")
            sfo2 = Pp.tile([2, R], F32, name="sfo2")
            pf2o = PsS.tile([2, 512], F32, tag="ps_s", bufs=2)
            for fc in range(FC):
                nc.tensor.matmul(
                    pf2o[:], sw2[:, fc, :], sxcb[:, fc, :],
                    start=(fc == 0), stop=(fc == 3))
            nc.vector.tensor_copy(sfo2[:], pf2o[:])
            nc.sync.dma_start(dblk2b[:], sfo2[1:2, :])
            for nc4 in range(4):
                pH = PsA.tile([128, 512], F32, tag="ps_a", bufs=2)
                for fc in range(FC):
                    nc.tensor.matmul(
                        pH[:, 0:NCLASS],
                        sxcb[:, fc, nc4 * 128:(nc4 + 1) * 128],
                        sWout[:, fc, :],
                        start=(fc == 0), stop=(fc == 3))
                sh2b = Pp.tile([128, NCLASS], F32, tag="sh2b", bufs=2)
                nc.vector.tensor_copy(sh2b[:], pH[:, 0:NCLASS])
                nc.sync.dma_start(
                    dblk2.ap()[nc4 * 128:(nc4 + 1) * 128, :]
                    if False else dblk2[nc4 * 128:(nc4 + 1) * 128, :],
                    sh2b[:])
            nc.gpsimd.collective_compute(
                "AllGather", ALU.bypass,
                replica_groups=[list(range(NCORES))],
                ins=[dblk2b[:].opt()], outs=[dgath2b[:].opt()])
            nc.gpsimd.collective_compute(
                "AllGather", ALU.bypass,
                replica_groups=[list(range(NCORES))],
                ins=[dblk2[:].opt()], outs=[dgath2[:].opt()])

            # ======================== layer 2 ========================
            with tc.tile_pool(name="stage2", bufs=1) as P2:
                sh2r = P2.tile([128, JC, NCLASS], F32R, name="sh2r")
                for jc in range(JC):
                    nc.sync.dma_start(
                        sh2r[:, jc, :],
                        dgath2[jc * 128:(jc + 1) * 128, :].bitcast(F32R))
                ident = P2.tile([128, 128], F32, name="ident")
                nc.sync.dma_start(ident[:], id_d.ap())
                sfj2T = P2.tile([128, JC], F32, name="sfj2T")
                nc.sync.dma_start(
                    sfj2T[:].rearrange("p (r jc) -> p r jc", r=8),
                    dgath2b[:].rearrange("r (jc p) -> p r jc", p=128))
                fib2 = P2.tile([128, R], F32, name="fib2")
                nc.gpsimd.partition_broadcast(fib2[:], sfo2[0:1, :])

                # layer-2 attention chunks (batch 4 jc per Exp)
                pagg2 = Pagg.tile([128, 512], F32, tag="agg0", bufs=1)
                pZ2 = Pagg.tile([1, 512], F32, tag="agg1", bufs=1)
                for jb in range(8):
                    raw4 = P2.tile([128, 2048], F32, tag="raw4b", bufs=3)
                    em4 = P2.tile([128, 2048], F32, tag="em4b", bufs=3)
                    P4 = P2.tile([128, 2048], F32R, tag="p4b", bufs=8)
                    for q in range(4):
                        jc = jb * 4 + q
                        sl = slice(q * 512, (q + 1) * 512)
                        mask = P2.tile([128, 512], BF16, tag="maskb", bufs=3)
                        nc.sync.dma_start(
                            mask[:], adj_d.ap()[jc * 128:(jc + 1) * 128, :])
                        nc.vector.scalar_tensor_tensor(
                            raw4[:, sl], mask[:], BIG, fib2[:],
                            op0=ALU.mult, op1=ALU.add)
                        if (jc * 3) % 8 < 3:
                            u = P2.tile([128, 512], F32, tag="ulk2", bufs=3)
                            nc.vector.tensor_scalar_add(
                                u[:], raw4[:, sl], sfj2T[:, jc:jc + 1])
                            nc.vector.scalar_tensor_tensor(
                                em4[:, sl], u[:], ALPHA, u[:],
                                op0=ALU.mult, op1=ALU.max)
                        else:
                            nc.scalar.activation(
                                em4[:, sl], raw4[:, sl], AF.Prelu,
                                bias=sfj2T[:, jc:jc + 1], alpha=alpha[:])
                    nc.scalar.activation(P4[:], em4[:], AF.Exp)
                    for q in range(4):
                        jc = jb * 4 + q
                        sl = slice(q * 512, (q + 1) * 512)
                        nc.tensor.matmul(
                            pagg2[:], sh2r[:, jc, :], P4[:, sl],
                            start=(jc == 0), stop=(jc == JC - 1))
                        nc.tensor.matmul(
                            pZ2[:], onescol[:], P4[:, sl],
                            start=(jc == 0), stop=(jc == JC - 1))

                # normalize, elu, transpose, log_softmax
                sz2 = P2.tile([1, R], F32, name="sz2")
                nc.vector.tensor_copy(sz2[:], pZ2[0:1, :])
                srz2 = P2.tile([1, R], F32, name="srz2")
                nc.vector.reciprocal(srz2[:], sz2[:])
                zb2 = P2.tile([128, R], F32, name="zb2")
                nc.gpsimd.partition_broadcast(zb2[:], srz2[:])
                sv = P2.tile([128, R], F32, name="sv")
                nc.vector.tensor_mul(sv[:], pagg2[:], zb2[:])
                smin = P2.tile([128, R], F32, name="smin")
                nc.vector.tensor_scalar_min(smin[:], sv[:], 0.0)
                sex = P2.tile([128, R], F32, name="sex")
                nc.scalar.activation(sex[:], smin[:], AF.Exp)
                srel = P2.tile([128, R], F32, name="srel")
                nc.scalar.activation(srel[:], sv[:], AF.Relu)
                sres = P2.tile([128, R], F32, name="sres")
                nc.vector.scalar_tensor_tensor(
                    sres[:], sex[:], -1.0, srel[:], op0=ALU.add, op1=ALU.add)

                for it in range(4):
                    ptp = PsS.tile([128, 128], F32, tag="ps_s", bufs=2)
                    nc.tensor.transpose(
                        ptp[:], sres[:, it * 128:(it + 1) * 128], ident[:])
                    st = P2.tile([128, 128], F32, tag="st", bufs=2)
                    nc.vector.tensor_copy(st[:], ptp[:])
                    mx = P2.tile([128, 1], F32, tag="mx", bufs=2)
                    nc.vector.tensor_reduce(
                        mx[:], st[:], axis=mybir.AxisListType.X, op=ALU.max)
                    negmx = P2.tile([128, 1], F32, tag="negmx", bufs=2)
                    nc.vector.tensor_scalar_mul(negmx[:], mx[:], -1.0)
                    sexp = P2.tile([128, 128], F32, tag="sexp", bufs=2)
                    ssum = P2.tile([128, 1], F32, tag="ssum", bufs=2)
                    nc.scalar.activation(
                        sexp[:], st[:], AF.Exp, bias=negmx[:],
                        accum_out=ssum[:])
                    sln = P2.tile([128, 1], F32, tag="sln", bufs=2)
                    nc.scalar.activation(sln[:], ssum[:], AF.Ln)
                    b2 = P2.tile([128, 1], F32, tag="b2", bufs=2)
                    nc.vector.tensor_sub(b2[:], negmx[:], sln[:])
                    sout = P2.tile([128, 128], F32, tag="sout", bufs=2)
                    nc.scalar.activation(sout[:], st[:], AF.Identity, bias=b2[:])
                    nc.sync.dma_start(
                        out_d.ap()[it * 128:(it + 1) * 128, :], sout[:])

    nc.finalize()
    return nc


def _get_nc():
    if "nc" not in _CACHE:
        _CACHE["nc"] = _build_nc()
    return _CACHE["nc"]


def kernel(**inputs):
    x = np.asarray(inputs["x"], dtype=np.float32)
    adj = np.asarray(inputs["adj"])
    W = np.asarray(inputs["W"], dtype=np.float32)
    a = np.asarray(inputs["a"], dtype=np.float32)
    W_out = np.asarray(inputs["W_out"], dtype=np.float32)
    a_out = np.asarray(inputs["a_out"], dtype=np.float32)

    xT = np.ascontiguousarray(x.T)
    Wcat = np.ascontiguousarray(W.transpose(1, 0, 2).reshape(NFEAT, 512))
    WcatT = np.ascontiguousarray(Wcat.T)
    A12 = np.zeros((512, 16), np.float32)
    for hd in range(NHEADS):
        A12[hd * NHID:(hd + 1) * NHID, hd] = a[hd, NHID:]      # a2 -> fj
        A12[hd * NHID:(hd + 1) * NHID, 8 + hd] = a[hd, :NHID]  # a1 -> fi
    WoutT = np.ascontiguousarray(W_out.T)
    AO = np.stack([a_out[:NCLASS], a_out[NCLASS:]], axis=1)
    AO = np.ascontiguousarray(AO, dtype=np.float32)
    ident = np.eye(128, dtype=np.float32)
    adjm1 = adj.astype(np.float32) - 1.0

    in_maps = []
    for c in range(NCORES):
        r0, r1 = c * R, (c + 1) * R
        in_maps.append({
            "xT": xT,
            "xTblk": np.ascontiguousarray(x[r0:r1].T),
            "Wcat": Wcat,
            "WcatT": WcatT,
            "A12": A12,
            "Wout": W_out,
            "WoutT": WoutT,
            "AO": AO,
            "adjm1T": np.ascontiguousarray(adjm1[r0:r1].T).astype(
                ml_dtypes.bfloat16),
            "ident": ident,
        })

    nc = _get_nc()
    trace = bool(os.environ.get("KERNEL_TRACE"))
    res = bass_utils.run_bass_kernel_spmd(
        nc, in_maps, list(range(NCORES)), trace=trace)
    kernel.last_results = res
    out = np.concatenate(
        [res.results[c]["out"] for c in range(NCORES)], axis=0)
    return np.ascontiguousarray(out, dtype=np.float32)


# revision 16
# speedup vs baseline: 1.1593x; 1.1593x over previous
"""GAT (2-layer multi-head graph attention) on 8 Trainium2 NeuronCores.

Sharding: nodes (rows of adj / attention) are sharded across the 8 cores;
each core computes h = x@W replicated, its 512-row block of
e/softmax/aggregation for both GAT layers, with an AllGather of the layer-1
output (xcat) between layers.

Layout trick: attention probabilities are computed TRANSPOSED (eT[j, i]) so
softmax-normalizer and aggregation both run on the tensor engine:
  aggT[o, i] = sum_j hplus[j, o] * P[j, i]  with hplus = [h | 1] so the last
row of the accumulator is the softmax denominator Z.  exp/leaky run on the
scalar engine (Prelu alpha=0.2 + Exp share one ACT table set), masking is a
single DVE scalar_tensor_tensor using (adj-1)*100 added before the leaky
(masked entries land at ~exp(-16) -> 0).
"""
import os
import sys

for _p in ("/opt/trn_rl_repo", "/root/.axon_site/_ro/trn_rl_repo"):
    if os.path.isdir(_p) and _p not in sys.path:
        sys.path.insert(0, _p)

import numpy as np
import ml_dtypes

import concourse.bacc as bacc
import concourse.mybir as mybir
import concourse.tile as tile
from concourse import bass_utils

F32 = mybir.dt.float32
F32R = mybir.dt.float32r
BF16 = mybir.dt.bfloat16
AF = mybir.ActivationFunctionType
ALU = mybir.AluOpType

N, NFEAT, NHID, NCLASS, NHEADS = 4096, 512, 64, 128, 8
NCORES = 8
R = N // NCORES          # 512 rows per core
FC = NFEAT // 128        # 4 feature chunks
JC = N // 128            # 32 j-chunks
BIG = 100.0
ALPHA = 0.2

_CACHE = {}


def _build_nc():
    nc = bacc.Bacc("TRN2", target_bir_lowering=False, debug=False,
                   num_devices=NCORES)

    xT_d = nc.dram_tensor("xT", [NFEAT, N], F32R, kind="ExternalInput")
    xTb_d = nc.dram_tensor("xTblk", [NFEAT, R], F32R, kind="ExternalInput")
    Wcat_d = nc.dram_tensor("Wcat", [NFEAT, 512], F32R, kind="ExternalInput")
    WcatT_d = nc.dram_tensor("WcatT", [512, NFEAT], F32R, kind="ExternalInput")
    A12_d = nc.dram_tensor("A12", [512, 16], F32R, kind="ExternalInput")
    Wout_d = nc.dram_tensor("Wout", [512, NCLASS], F32R, kind="ExternalInput")
    WoutT_d = nc.dram_tensor("WoutT", [NCLASS, 512], F32R, kind="ExternalInput")
    AO_d = nc.dram_tensor("AO", [NCLASS, 2], F32R, kind="ExternalInput")
    adj_d = nc.dram_tensor("adjm1T", [N, R], BF16, kind="ExternalInput")
    id_d = nc.dram_tensor("ident", [128, 128], F32, kind="ExternalInput")
    out_d = nc.dram_tensor("out", [R, NCLASS], F32, kind="ExternalOutput")

    with tile.TileContext(nc, num_cores=NCORES) as tc:
        with (
            tc.tile_pool(name="persist", bufs=1) as Pp,
            tc.tile_pool(name="dram", bufs=1, space="DRAM") as Pd,
            tc.tile_pool(name="psA", bufs=2, space="PSUM") as PsA,
            tc.tile_pool(name="psS", bufs=2, space="PSUM") as PsS,
            tc.tile_pool(name="pagg", bufs=1, space="PSUM") as Pagg,
        ):
            # ---- persistent constants / small state ----
            alpha = Pp.tile([128, 1], F32, name="alpha")
            nc.vector.memset(alpha[:], ALPHA)
            onescol = Pp.tile([128, 1], F32R, name="onescol")
            nc.vector.memset(onescol[:].bitcast(F32), 1.0)
            sfjT = Pp.tile([128, JC, 8], F32, name="sfjT")
            sxcb = Pp.tile([128, FC, R], F32, name="sxcb")  # own xcatT block
            sw12 = Pp.tile([128, FC, 16], F32, name="sw12")
            sWout = Pp.tile([128, FC, NCLASS], F32, name="sWout")
            for fc in range(FC):
                nc.sync.dma_start(
                    sWout[:, fc, :],
                    Wout_d.ap()[fc * 128:(fc + 1) * 128, :].bitcast(F32))
            sWoutT = Pp.tile([128, 512], F32, name="sWoutT")
            nc.sync.dma_start(sWoutT[:], WoutT_d.ap().bitcast(F32))
            sAO = Pp.tile([128, 2], F32, name="sAO")
            nc.sync.dma_start(sAO[:], AO_d.ap().bitcast(F32))
            sw2 = Pp.tile([128, FC, 2], F32, name="sw2")
            for fc in range(FC):
                pw2 = PsS.tile([128, 2], F32, tag="ps_s", bufs=2)
                nc.tensor.matmul(
                    pw2[:], sWoutT[:, fc * 128:(fc + 1) * 128], sAO[:],
                    start=True, stop=True)
                nc.vector.tensor_copy(sw2[:, fc, :], pw2[:])
            fibs = [Pp.tile([128, R], F32, name=f"fib{h}") for h in range(NHEADS)]

            with tc.tile_pool(name="hplusp", bufs=1) as Ph:
                shplus = Ph.tile([128, JC, NHEADS, NHID + 1], F32R, name="shplus")
                nc.vector.memset(shplus[:, :, :, NHID].bitcast(F32), 1.0)

                # ================= stage 1: weights / fifj =================
                with tc.tile_pool(name="stage1", bufs=1) as P1:
                    sxT = P1.tile([128, FC, N], F32R, name="sxT")
                    for fc in range(FC):
                        nc.sync.dma_start(
                            sxT[:, fc, :], xT_d.ap()[fc * 128:(fc + 1) * 128, :])
                    sWcat = P1.tile([128, FC, 512], F32R, name="sWcat")
                    for fc in range(FC):
                        nc.sync.dma_start(
                            sWcat[:, fc, :], Wcat_d.ap()[fc * 128:(fc + 1) * 128, :])
                    sfown = P1.tile([16, R], F32, name="sfown")

                    with tc.tile_pool(name="stage1a", bufs=1) as P1a:
                        sA12 = P1a.tile([128, 4, 16], F32, name="sA12")
                        for hoc in range(4):
                            nc.sync.dma_start(
                                sA12[:, hoc, :],
                                A12_d.ap()[hoc * 128:(hoc + 1) * 128, :].bitcast(F32))
                        sxTb = P1a.tile([128, FC, R], F32, name="sxTb")
                        for fc in range(FC):
                            nc.sync.dma_start(
                                sxTb[:, fc, :],
                                xTb_d.ap()[fc * 128:(fc + 1) * 128, :].bitcast(F32))

                        # w12[f, k] = sum_ho WcatT[ho, f] * A12[ho, k]
                        # stream 128x128 WcatT tiles per (fc, hoc)
                        for fc in range(FC):
                            pw = PsS.tile([128, 16], F32, tag="ps_s", bufs=2)
                            for hoc in range(4):
                                swcT = P1a.tile([128, 128], F32, tag="swcT",
                                                bufs=2)
                                nc.sync.dma_start(
                                    swcT[:],
                                    WcatT_d.ap()[hoc * 128:(hoc + 1) * 128,
                                                 fc * 128:(fc + 1) * 128]
                                    .bitcast(F32))
                                nc.tensor.matmul(
                                    pw[:], swcT[:], sA12[:, hoc, :],
                                    start=(hoc == 0), stop=(hoc == 3))
                            nc.vector.tensor_copy(sw12[:, fc, :], pw[:])

                        # fj columns directly (fp32), 4 jc per streamed tile
                        for jcg in range(8):
                            xtf = []
                            for fc in range(FC):
                                t = P1a.tile([128, 512], F32, tag=f"xtf{fc}",
                                             bufs=2, name=f"xtf{fc}_{jcg}")
                                nc.sync.dma_start(
                                    t[:],
                                    xT_d.ap()[fc * 128:(fc + 1) * 128,
                                              jcg * 512:(jcg + 1) * 512]
                                    .bitcast(F32))
                                xtf.append(t)
                            for q in range(4):
                                jc = jcg * 4 + q
                                pfj = PsS.tile([128, 8], F32, tag="ps_s",
                                               bufs=2)
                                for fc in range(FC):
                                    nc.tensor.matmul(
                                        pfj[:],
                                        xtf[fc][:, q * 128:(q + 1) * 128],
                                        sw12[:, fc, 0:8],
                                        start=(fc == 0), stop=(fc == 3))
                                nc.vector.tensor_copy(sfjT[:, jc, :], pfj[:])
                        # own-block fifj (for fi of this core's rows)
                        pfo = PsS.tile([16, 512], F32, tag="ps_s", bufs=2)
                        for fc in range(FC):
                            nc.tensor.matmul(
                                pfo[:], sw12[:, fc, :], sxTb[:, fc, :],
                                start=(fc == 0), stop=(fc == 3))
                        nc.vector.tensor_copy(sfown[:], pfo[:])

                    # fib[h] = broadcast of own-block fi row (partition 0 src)
                    for hd in range(NHEADS):
                        fot = Pp.tile([1, R], F32, tag="fot", bufs=2)
                        nc.sync.dma_start(fot[:], sfown[8 + hd:9 + hd, :])
                        nc.gpsimd.partition_broadcast(fibs[hd][:], fot[:])

                    # ============ stage A: hcat natural -> hplus ============
                    for ncx in range(JC):
                        pA = PsA.tile([128, 512], F32, tag="ps_a", bufs=2)
                        for fc in range(FC):
                            nc.tensor.matmul(
                                pA[:], sxT[:, fc, ncx * 128:(ncx + 1) * 128],
                                sWcat[:, fc, :],
                                start=(fc == 0), stop=(fc == 3))
                        nc.scalar.copy(
                            shplus[:, ncx, :, 0:NHID],
                            pA[:].rearrange("p (hd o) -> p hd o", o=NHID))

                # ================= layer-1 attention sweeps =================
                with tc.tile_pool(name="chunkL1", bufs=1) as Pc:
                    paggs = {}
                    for sweep in range(2):
                        heads = list(range(sweep * 4, sweep * 4 + 4))
                        for jc in range(JC):
                            mask = Pc.tile([128, 512], BF16, tag="mask", bufs=3)
                            nc.sync.dma_start(
                                mask[:], adj_d.ap()[jc * 128:(jc + 1) * 128, :])
                            raw4 = Pc.tile([128, 2048], F32, tag="raw4", bufs=2)
                            em4 = Pc.tile([128, 2048], F32, tag="em4", bufs=2)
                            P4 = Pc.tile([128, 2048], F32R, tag="p4", bufs=2)
                            for q, hd in enumerate(heads):
                                sl = slice(q * 512, (q + 1) * 512)
                                gidx = (sweep * JC + jc) * 4 + q
                                nc.vector.scalar_tensor_tensor(
                                    raw4[:, sl], mask[:], BIG, fibs[hd][:],
                                    op0=ALU.mult, op1=ALU.add)
                                if (gidx * 3) % 8 < 3:
                                    u = Pc.tile([128, 512], F32, tag="ulk",
                                                bufs=3)
                                    nc.vector.tensor_scalar_add(
                                        u[:], raw4[:, sl],
                                        sfjT[:, jc, hd:hd + 1])
                                    nc.vector.scalar_tensor_tensor(
                                        em4[:, sl], u[:], ALPHA, u[:],
                                        op0=ALU.mult, op1=ALU.max)
                                else:
                                    nc.scalar.activation(
                                        em4[:, sl], raw4[:, sl], AF.Prelu,
                                        bias=sfjT[:, jc, hd:hd + 1],
                                        alpha=alpha[:])
                            nc.scalar.activation(P4[:], em4[:], AF.Exp)
                            for q, hd in enumerate(heads):
                                if jc == 0:
                                    paggs[hd] = Pagg.tile(
                                        [NHID + 1, 512], F32, tag=f"agg{q}",
                                        bufs=1, name=f"agg_s{sweep}_{q}")
                                nc.tensor.matmul(
                                    paggs[hd][:], shplus[:, jc, hd, :],
                                    P4[:, q * 512:(q + 1) * 512],
                                    start=(jc == 0), stop=(jc == JC - 1))

                        # normalize this sweep's heads into the xcatT block
                        zsw = Pc.tile([4, R], F32, tag="zsw", bufs=2)
                        for q, hd in enumerate(heads):
                            zst = Pc.tile([NHID + 1, R], F32, tag="zst", bufs=2)
                            nc.vector.tensor_copy(
                                zst[NHID:NHID + 1, :], paggs[hd][NHID:NHID + 1, :])
                            nc.sync.dma_start(
                                zsw[q:q + 1, :], zst[NHID:NHID + 1, :])
                        rzw = Pc.tile([4, R], F32, tag="rzw", bufs=2)
                        nc.vector.reciprocal(rzw[:], zsw[:])
                        for q, hd in enumerate(heads):
                            rzt = Pc.tile([1, R], F32, tag="rzt", bufs=2)
                            nc.sync.dma_start(rzt[:], rzw[q:q + 1, :])
                            zb = Pc.tile([64, R], F32, tag="zb", bufs=2)
                            nc.gpsimd.partition_broadcast(zb[:], rzt[:])
                            xcn = Pc.tile([64, R], F32, tag="xcn", bufs=2)
                            nc.vector.tensor_mul(
                                xcn[:], paggs[hd][0:NHID, :], zb[:])
                            nc.sync.dma_start(
                                sxcb[64 * (hd % 2):64 * (hd % 2) + 64,
                                     hd // 2, :], xcn[:])

            # ===== layer-2 projections on the OWN block, then small gather =====
            # h2_block[n, c] = sum_f xcat_blk[n, f] Wout[f, c]   (own 512 nodes)
            # fifj2_block = w2.T @ xcat_blkT  -> fi2 (row 0, local), fj2 (row 1)
            dblk2 = Pd.tile([R, NCLASS], F32, name="dblk2")
            dgath2 = Pd.tile([N, NCLASS], F32, name="dgath2",
                             addr_space="Shared")
            dblk2b = Pd.tile([1, R], F32, name="dblk2b")
            dgath2b = Pd.tile([8, R], F32, name="dgath2b",
                              addr_space="Shared")
            sfo2 = Pp.tile([2, R], F32, name="sfo2")
            pf2o = PsS.tile([2, 512], F32, tag="ps_s", bufs=2)
            for fc in range(FC):
                nc.tensor.matmul(
                    pf2o[:], sw2[:, fc, :], sxcb[:, fc, :],
                    start=(fc == 0), stop=(fc == 3))
            nc.vector.tensor_copy(sfo2[:], pf2o[:])
            nc.sync.dma_start(dblk2b[:], sfo2[1:2, :])
            for nc4 in range(4):
                pH = PsA.tile([128, 512], F32, tag="ps_a", bufs=2)
                for fc in range(FC):
                    nc.tensor.matmul(
                        pH[:, 0:NCLASS],
                        sxcb[:, fc, nc4 * 128:(nc4 + 1) * 128],
                        sWout[:, fc, :],
                        start=(fc == 0), stop=(fc == 3))
                sh2b = Pp.tile([128, NCLASS], F32, tag="sh2b", bufs=2)
                nc.vector.tensor_copy(sh2b[:], pH[:, 0:NCLASS])
                nc.sync.dma_start(
                    dblk2.ap()[nc4 * 128:(nc4 + 1) * 128, :]
                    if False else dblk2[nc4 * 128:(nc4 + 1) * 128, :],
                    sh2b[:])
            nc.gpsimd.collective_compute(
                "AllGather", ALU.bypass,
                replica_groups=[list(range(NCORES))],
                ins=[dblk2b[:].opt()], outs=[dgath2b[:].opt()])
            nc.gpsimd.collective_compute(
                "AllGather", ALU.bypass,
                replica_groups=[list(range(NCORES))],
                ins=[dblk2[:].opt()], outs=[dgath2[:].opt()])

            # ======================== layer 2 ========================
            with tc.tile_pool(name="stage2", bufs=1) as P2:
                sh2r = P2.tile([128, JC, NCLASS], F32R, name="sh2r")
                for jc in range(JC):
                    nc.sync.dma_start(
                        sh2r[:, jc, :],
                        dgath2[jc * 128:(jc + 1) * 128, :].bitcast(F32R))
                ident = P2.tile([128, 128], F32, name="ident")
                nc.sync.dma_start(ident[:], id_d.ap())
                sfj2T = P2.tile([128, JC], F32, name="sfj2T")
                nc.sync.dma_start(
                    sfj2T[:].rearrange("p (r jc) -> p r jc", r=8),
                    dgath2b[:].rearrange("r (jc p) -> p r jc", p=128))
                fib2 = P2.tile([128, R], F32, name="fib2")
                nc.gpsimd.partition_broadcast(fib2[:], sfo2[0:1, :])

                # layer-2 attention chunks (batch 4 jc per Exp)
                pagg2 = Pagg.tile([128, 512], F32, tag="agg0", bufs=1)
                pZ2 = Pagg.tile([1, 512], F32, tag="agg1", bufs=1)
                for jb in range(8):
                    raw4 = P2.tile([128, 2048], F32, tag="raw4b", bufs=3)
                    em4 = P2.tile([128, 2048], F32, tag="em4b", bufs=3)
                    P4 = P2.tile([128, 2048], F32R, tag="p4b", bufs=8)
                    for q in range(4):
                        jc = jb * 4 + q
                        sl = slice(q * 512, (q + 1) * 512)
                        mask = P2.tile([128, 512], BF16, tag="maskb", bufs=3)
                        nc.sync.dma_start(
                            mask[:], adj_d.ap()[jc * 128:(jc + 1) * 128, :])
                        nc.vector.scalar_tensor_tensor(
                            raw4[:, sl], mask[:], BIG, fib2[:],
                            op0=ALU.mult, op1=ALU.add)
                        if (jc * 3) % 8 < 3:
                            u = P2.tile([128, 512], F32, tag="ulk2", bufs=3)
                            nc.vector.tensor_scalar_add(
                                u[:], raw4[:, sl], sfj2T[:, jc:jc + 1])
                            nc.vector.scalar_tensor_tensor(
                                em4[:, sl], u[:], ALPHA, u[:],
                                op0=ALU.mult, op1=ALU.max)
                        else:
                            nc.scalar.activation(
                                em4[:, sl], raw4[:, sl], AF.Prelu,
                                bias=sfj2T[:, jc:jc + 1], alpha=alpha[:])
                    nc.scalar.activation(P4[:], em4[:], AF.Exp)
                    for q in range(4):
                        jc = jb * 4 + q
                        sl = slice(q * 512, (q + 1) * 512)
                        nc.tensor.matmul(
                            pagg2[:], sh2r[:, jc, :], P4[:, sl],
                            start=(jc == 0), stop=(jc == JC - 1))
                        nc.tensor.matmul(
                            pZ2[:], onescol[:], P4[:, sl],
                            start=(jc == 0), stop=(jc == JC - 1))

                # normalize, elu, transpose, log_softmax
                sz2 = P2.tile([1, R], F32, name="sz2")
                nc.vector.tensor_copy(sz2[:], pZ2[0:1, :])
                srz2 = P2.tile([1, R], F32, name="srz2")
                nc.vector.reciprocal(srz2[:], sz2[:])
                zb2 = P2.tile([128, R], F32, name="zb2")
                nc.gpsimd.partition_broadcast(zb2[:], srz2[:])
                sv = P2.tile([128, R], F32, name="sv")
                nc.vector.tensor_mul(sv[:], pagg2[:], zb2[:])
                smin = P2.tile([128, R], F32, name="smin")
                nc.vector.tensor_scalar_min(smin[:], sv[:], 0.0)
                sex = P2.tile([128, R], F32, name="sex")
                nc.scalar.activation(sex[:], smin[:], AF.Exp)
                srel = P2.tile([128, R], F32, name="srel")
                nc.scalar.activation(srel[:], sv[:], AF.Relu)
                sres = P2.tile([128, R], F32, name="sres")
                nc.vector.scalar_tensor_tensor(
                    sres[:], sex[:], -1.0, srel[:], op0=ALU.add, op1=ALU.add)

                for it in range(4):
                    ptp = PsS.tile([128, 128], F32, tag="ps_s", bufs=2)
                    nc.tensor.transpose(
                        ptp[:], sres[:, it * 128:(it + 1) * 128], ident[:])
                    st = P2.tile([128, 128], F32, tag="st", bufs=2)
                    nc.vector.tensor_copy(st[:], ptp[:])
                    mx = P2.tile([128, 1], F32, tag="mx", bufs=2)
                    nc.vector.tensor_reduce(
                        mx[:], st[:], axis=mybir.AxisListType.X, op=ALU.max)
                    negmx = P2.tile([128, 1], F32, tag="negmx", bufs=2)
                    nc.vector.tensor_scalar_mul(negmx[:], mx[:], -1.0)
                    sexp = P2.tile([128, 128], F32, tag="sexp", bufs=2)
                    ssum = P2.tile([128, 1], F32, tag="ssum", bufs=2)
                    nc.scalar.activation(
                        sexp[:], st[:], AF.Exp, bias=negmx[:],
                        accum_out=ssum[:])
                    sln = P2.tile([128, 1], F32, tag="sln", bufs=2)
                    nc.scalar.activation(sln[:], ssum[:], AF.Ln)
                    b2 = P2.tile([128, 1], F32, tag="b2", bufs=2)
                    nc.vector.tensor_sub(b2[:], negmx[:], sln[:])
                    sout = P2.tile([128, 128], F32, tag="sout", bufs=2)
                    nc.scalar.activation(sout[:], st[:], AF.Identity, bias=b2[:])
                    nc.sync.dma_start(
                        out_d.ap()[it * 128:(it + 1) * 128, :], sout[:])

    nc.finalize()
    return nc


def _get_nc():
    if "nc" not in _CACHE:
        _CACHE["nc"] = _build_nc()
    return _CACHE["nc"]


def kernel(**inputs):
    x = np.asarray(inputs["x"], dtype=np.float32)
    adj = np.asarray(inputs["adj"])
    W = np.asarray(inputs["W"], dtype=np.float32)
    a = np.asarray(inputs["a"], dtype=np.float32)
    W_out = np.asarray(inputs["W_out"], dtype=np.float32)
    a_out = np.asarray(inputs["a_out"], dtype=np.float32)

    xT = np.ascontiguousarray(x.T)
    Wcat = np.ascontiguousarray(W.transpose(1, 0, 2).reshape(NFEAT, 512))
    WcatT = np.ascontiguousarray(Wcat.T)
    A12 = np.zeros((512, 16), np.float32)
    for hd in range(NHEADS):
        A12[hd * NHID:(hd + 1) * NHID, hd] = a[hd, NHID:]      # a2 -> fj
        A12[hd * NHID:(hd + 1) * NHID, 8 + hd] = a[hd, :NHID]  # a1 -> fi
    WoutT = np.ascontiguousarray(W_out.T)
    AO = np.stack([a_out[:NCLASS], a_out[NCLASS:]], axis=1)
    AO = np.ascontiguousarray(AO, dtype=np.float32)
    ident = np.eye(128, dtype=np.float32)
    adjm1 = adj.astype(np.float32) - 1.0

    in_maps = []
    for c in range(NCORES):
        r0, r1 = c * R, (c + 1) * R
        in_maps.append({
            "xT": xT,
            "xTblk": np.ascontiguousarray(x[r0:r1].T),
            "Wcat": Wcat,
            "WcatT": WcatT,
            "A12": A12,
            "Wout": W_out,
            "WoutT": WoutT,
            "AO": AO,
            "adjm1T": np.ascontiguousarray(adjm1[r0:r1].T).astype(
                ml_dtypes.bfloat16),
            "ident": ident,
        })

    nc = _get_nc()
    trace = bool(os.environ.get("KERNEL_TRACE"))
    res = bass_utils.run_bass_kernel_spmd(
        nc, in_maps, list(range(NCORES)), trace=trace)
    kernel.last_results = res
    out = np.concatenate(
        [res.results[c]["out"] for c in range(NCORES)], axis=0)
    return np.ascontiguousarray(out, dtype=np.float32)


# revision 17
# speedup vs baseline: 1.2090x; 1.0428x over previous
"""GAT (2-layer multi-head graph attention) on 8 Trainium2 NeuronCores.

Sharding: nodes (rows of adj / attention) are sharded across the 8 cores;
each core computes h = x@W replicated, its 512-row block of
e/softmax/aggregation for both GAT layers, with an AllGather of the layer-1
output (xcat) between layers.

Layout trick: attention probabilities are computed TRANSPOSED (eT[j, i]) so
softmax-normalizer and aggregation both run on the tensor engine:
  aggT[o, i] = sum_j hplus[j, o] * P[j, i]  with hplus = [h | 1] so the last
row of the accumulator is the softmax denominator Z.  exp/leaky run on the
scalar engine (Prelu alpha=0.2 + Exp share one ACT table set), masking is a
single DVE scalar_tensor_tensor using (adj-1)*100 added before the leaky
(masked entries land at ~exp(-16) -> 0).
"""
import os
import sys

for _p in ("/opt/trn_rl_repo", "/root/.axon_site/_ro/trn_rl_repo"):
    if os.path.isdir(_p) and _p not in sys.path:
        sys.path.insert(0, _p)

import numpy as np
import ml_dtypes

import concourse.bacc as bacc
import concourse.mybir as mybir
import concourse.tile as tile
from concourse import bass_utils

F32 = mybir.dt.float32
F32R = mybir.dt.float32r
BF16 = mybir.dt.bfloat16
AF = mybir.ActivationFunctionType
ALU = mybir.AluOpType

N, NFEAT, NHID, NCLASS, NHEADS = 4096, 512, 64, 128, 8
NCORES = 8
R = N // NCORES          # 512 rows per core
FC = NFEAT // 128        # 4 feature chunks
JC = N // 128            # 32 j-chunks
BIG = 100.0
ALPHA = 0.2

_CACHE = {}


def _build_nc():
    nc = bacc.Bacc("TRN2", target_bir_lowering=False, debug=False,
                   num_devices=NCORES)

    xT_d = nc.dram_tensor("xT", [NFEAT, N], F32R, kind="ExternalInput")
    xTb_d = nc.dram_tensor("xTblk", [NFEAT, R], F32R, kind="ExternalInput")
    Wcat_d = nc.dram_tensor("Wcat", [NFEAT, 512], F32R, kind="ExternalInput")
    WcatT_d = nc.dram_tensor("WcatT", [512, NFEAT], F32R, kind="ExternalInput")
    A12_d = nc.dram_tensor("A12", [512, 16], F32R, kind="ExternalInput")
    Wout_d = nc.dram_tensor("Wout", [512, NCLASS], F32R, kind="ExternalInput")
    WoutT_d = nc.dram_tensor("WoutT", [NCLASS, 512], F32R, kind="ExternalInput")
    AO_d = nc.dram_tensor("AO", [NCLASS, 2], F32R, kind="ExternalInput")
    adj_d = nc.dram_tensor("adjm1T", [N, R], BF16, kind="ExternalInput")
    id_d = nc.dram_tensor("ident", [128, 128], F32, kind="ExternalInput")
    out_d = nc.dram_tensor("out", [R, NCLASS], F32, kind="ExternalOutput")

    with tile.TileContext(nc, num_cores=NCORES) as tc:
        with (
            tc.tile_pool(name="persist", bufs=1) as Pp,
            tc.tile_pool(name="dram", bufs=1, space="DRAM") as Pd,
            tc.tile_pool(name="psA", bufs=2, space="PSUM") as PsA,
            tc.tile_pool(name="psS", bufs=2, space="PSUM") as PsS,
            tc.tile_pool(name="pagg", bufs=1, space="PSUM") as Pagg,
        ):
            # ---- persistent constants / small state ----
            alpha = Pp.tile([128, 1], F32, name="alpha")
            nc.vector.memset(alpha[:], ALPHA)
            onescol = Pp.tile([128, 1], F32R, name="onescol")
            nc.vector.memset(onescol[:].bitcast(F32), 1.0)
            sfjT = Pp.tile([128, JC, 8], F32, name="sfjT")
            sxcb = Pp.tile([128, FC, R], F32, name="sxcb")  # own xcatT block
            sw12 = Pp.tile([128, FC, 16], F32, name="sw12")
            sWcatF = Pp.tile([128, FC, 512], F32, name="sWcatF")
            for fc in range(FC):
                nc.sync.dma_start(
                    sWcatF[:, fc, :],
                    Wcat_d.ap()[fc * 128:(fc + 1) * 128, :].bitcast(F32))
            sWout = Pp.tile([128, FC, NCLASS], F32, name="sWout")
            for fc in range(FC):
                nc.sync.dma_start(
                    sWout[:, fc, :],
                    Wout_d.ap()[fc * 128:(fc + 1) * 128, :].bitcast(F32))
            sWoutT = Pp.tile([128, 512], F32, name="sWoutT")
            nc.sync.dma_start(sWoutT[:], WoutT_d.ap().bitcast(F32))
            sAO = Pp.tile([128, 2], F32, name="sAO")
            nc.sync.dma_start(sAO[:], AO_d.ap().bitcast(F32))
            sw2 = Pp.tile([128, FC, 2], F32, name="sw2")
            for fc in range(FC):
                pw2 = PsS.tile([128, 2], F32, tag="ps_s", bufs=2)
                nc.tensor.matmul(
                    pw2[:], sWoutT[:, fc * 128:(fc + 1) * 128], sAO[:],
                    start=True, stop=True)
                nc.vector.tensor_copy(sw2[:, fc, :], pw2[:])
            fibs = [Pp.tile([128, R], F32, name=f"fib{h}") for h in range(NHEADS)]

            with tc.tile_pool(name="hplusp", bufs=1) as Ph:
                shplus = Ph.tile([128, JC, NHEADS, NHID + 1], F32R, name="shplus")
                nc.vector.memset(shplus[:, :, :, NHID].bitcast(F32), 1.0)

                # ================= stage 1: weights / fifj =================
                with tc.tile_pool(name="stage1", bufs=1) as P1:
                    sfown = P1.tile([16, R], F32, name="sfown")

                    with tc.tile_pool(name="stage1a", bufs=1) as P1a:
                        sA12 = P1a.tile([128, 4, 16], F32, name="sA12")
                        for hoc in range(4):
                            nc.sync.dma_start(
                                sA12[:, hoc, :],
                                A12_d.ap()[hoc * 128:(hoc + 1) * 128, :].bitcast(F32))
                        sxTb = P1a.tile([128, FC, R], F32, name="sxTb")
                        for fc in range(FC):
                            nc.sync.dma_start(
                                sxTb[:, fc, :],
                                xTb_d.ap()[fc * 128:(fc + 1) * 128, :].bitcast(F32))

                        # w12[f, k] = sum_ho WcatT[ho, f] * A12[ho, k]
                        # stream 128x128 WcatT tiles per (fc, hoc)
                        for fc in range(FC):
                            pw = PsS.tile([128, 16], F32, tag="ps_s", bufs=2)
                            for hoc in range(4):
                                swcT = P1a.tile([128, 128], F32, tag="swcT",
                                                bufs=2)
                                nc.sync.dma_start(
                                    swcT[:],
                                    WcatT_d.ap()[hoc * 128:(hoc + 1) * 128,
                                                 fc * 128:(fc + 1) * 128]
                                    .bitcast(F32))
                                nc.tensor.matmul(
                                    pw[:], swcT[:], sA12[:, hoc, :],
                                    start=(hoc == 0), stop=(hoc == 3))
                            nc.vector.tensor_copy(sw12[:, fc, :], pw[:])

                        # own-block fifj (for fi of this core's rows)
                        pfo = PsS.tile([16, 512], F32, tag="ps_s", bufs=2)
                        for fc in range(FC):
                            nc.tensor.matmul(
                                pfo[:], sw12[:, fc, :], sxTb[:, fc, :],
                                start=(fc == 0), stop=(fc == 3))
                        nc.vector.tensor_copy(sfown[:], pfo[:])

                    # fib[h] = broadcast of own-block fi row (partition 0 src)
                    for hd in range(NHEADS):
                        fot = Pp.tile([1, R], F32, tag="fot", bufs=2)
                        nc.sync.dma_start(fot[:], sfown[8 + hd:9 + hd, :])
                        nc.gpsimd.partition_broadcast(fibs[hd][:], fot[:])

                # ================= layer-1 attention sweeps =================
                with tc.tile_pool(name="chunkL1", bufs=1) as Pc:
                    def prep_jc(jc):
                        """stage-A hplus[jc] + fj columns[jc], exact fp32,
                        streaming x tiles from DRAM."""
                        xa = []
                        for fc in range(FC):
                            t = Pc.tile([128, 128], F32, tag=f"xa{fc}",
                                        bufs=2, name=f"xa{fc}_{jc}")
                            nc.sync.dma_start(
                                t[:], xT_d.ap()[fc * 128:(fc + 1) * 128,
                                                jc * 128:(jc + 1) * 128]
                                .bitcast(F32))
                            xa.append(t)
                        pA = PsA.tile([128, 512], F32, tag="ps_a", bufs=2,
                                      name=f"pA{jc}")
                        for fc in range(FC):
                            nc.tensor.matmul(
                                pA[:], xa[fc][:], sWcatF[:, fc, :],
                                start=(fc == 0), stop=(fc == 3))
                        nc.scalar.copy(
                            shplus[:, jc, :, 0:NHID],
                            pA[:].rearrange("p (hd o) -> p hd o", o=NHID))
                        pfj = PsS.tile([128, 8], F32, tag="ps_s", bufs=2,
                                       name=f"pfj{jc}")
                        for fc in range(FC):
                            nc.tensor.matmul(
                                pfj[:], xa[fc][:], sw12[:, fc, 0:8],
                                start=(fc == 0), stop=(fc == 3))
                        nc.vector.tensor_copy(sfjT[:, jc, :], pfj[:])

                    prep_jc(0)
                    prep_jc(1)
                    paggs = {}
                    for sweep in range(2):
                        heads = list(range(sweep * 4, sweep * 4 + 4))
                        for jc in range(JC):
                            if sweep == 0 and jc + 2 < JC:
                                prep_jc(jc + 2)
                            mask = Pc.tile([128, 512], BF16, tag="mask", bufs=3)
                            nc.sync.dma_start(
                                mask[:], adj_d.ap()[jc * 128:(jc + 1) * 128, :])
                            raw4 = Pc.tile([128, 2048], F32, tag="raw4", bufs=2)
                            em4 = Pc.tile([128, 2048], F32, tag="em4", bufs=2)
                            P4 = Pc.tile([128, 2048], F32R, tag="p4", bufs=2)
                            for q, hd in enumerate(heads):
                                sl = slice(q * 512, (q + 1) * 512)
                                gidx = (sweep * JC + jc) * 4 + q
                                nc.vector.scalar_tensor_tensor(
                                    raw4[:, sl], mask[:], BIG, fibs[hd][:],
                                    op0=ALU.mult, op1=ALU.add)
                                if (gidx * 3) % 8 < 3:
                                    u = Pc.tile([128, 512], F32, tag="ulk",
                                                bufs=3)
                                    nc.vector.tensor_scalar_add(
                                        u[:], raw4[:, sl],
                                        sfjT[:, jc, hd:hd + 1])
                                    nc.vector.scalar_tensor_tensor(
                                        em4[:, sl], u[:], ALPHA, u[:],
                                        op0=ALU.mult, op1=ALU.max)
                                else:
                                    nc.scalar.activation(
                                        em4[:, sl], raw4[:, sl], AF.Prelu,
                                        bias=sfjT[:, jc, hd:hd + 1],
                                        alpha=alpha[:])
                            nc.scalar.activation(P4[:], em4[:], AF.Exp)
                            for q, hd in enumerate(heads):
                                if jc == 0:
                                    paggs[hd] = Pagg.tile(
                                        [NHID + 1, 512], F32, tag=f"agg{q}",
                                        bufs=1, name=f"agg_s{sweep}_{q}")
                                nc.tensor.matmul(
                                    paggs[hd][:], shplus[:, jc, hd, :],
                                    P4[:, q * 512:(q + 1) * 512],
                                    start=(jc == 0), stop=(jc == JC - 1))

                        # normalize this sweep's heads into the xcatT block
                        zsw = Pc.tile([4, R], F32, tag="zsw", bufs=2)
                        for q, hd in enumerate(heads):
                            zst = Pc.tile([NHID + 1, R], F32, tag="zst", bufs=2)
                            nc.vector.tensor_copy(
                                zst[NHID:NHID + 1, :], paggs[hd][NHID:NHID + 1, :])
                            nc.sync.dma_start(
                                zsw[q:q + 1, :], zst[NHID:NHID + 1, :])
                        rzw = Pc.tile([4, R], F32, tag="rzw", bufs=2)
                        nc.vector.reciprocal(rzw[:], zsw[:])
                        for q, hd in enumerate(heads):
                            rzt = Pc.tile([1, R], F32, tag="rzt", bufs=2)
                            nc.sync.dma_start(rzt[:], rzw[q:q + 1, :])
                            zb = Pc.tile([64, R], F32, tag="zb", bufs=2)
                            nc.gpsimd.partition_broadcast(zb[:], rzt[:])
                            xcn = Pc.tile([64, R], F32, tag="xcn", bufs=2)
                            nc.vector.tensor_mul(
                                xcn[:], paggs[hd][0:NHID, :], zb[:])
                            nc.sync.dma_start(
                                sxcb[64 * (hd % 2):64 * (hd % 2) + 64,
                                     hd // 2, :], xcn[:])

            # ===== layer-2 projections on the OWN block, then small gather =====
            # h2_block[n, c] = sum_f xcat_blk[n, f] Wout[f, c]   (own 512 nodes)
            # fifj2_block = w2.T @ xcat_blkT  -> fi2 (row 0, local), fj2 (row 1)
            dblk2 = Pd.tile([R, NCLASS], F32, name="dblk2")
            dgath2 = Pd.tile([N, NCLASS], F32, name="dgath2",
                             addr_space="Shared")
            dblk2b = Pd.tile([1, R], F32, name="dblk2b")
            dgath2b = Pd.tile([8, R], F32, name="dgath2b",
                              addr_space="Shared")
            sfo2 = Pp.tile([2, R], F32, name="sfo2")
            pf2o = PsS.tile([2, 512], F32, tag="ps_s", bufs=2)
            for fc in range(FC):
                nc.tensor.matmul(
                    pf2o[:], sw2[:, fc, :], sxcb[:, fc, :],
                    start=(fc == 0), stop=(fc == 3))
            nc.vector.tensor_copy(sfo2[:], pf2o[:])
            nc.sync.dma_start(dblk2b[:], sfo2[1:2, :])
            for nc4 in range(4):
                pH = PsA.tile([128, 512], F32, tag="ps_a", bufs=2)
                for fc in range(FC):
                    nc.tensor.matmul(
                        pH[:, 0:NCLASS],
                        sxcb[:, fc, nc4 * 128:(nc4 + 1) * 128],
                        sWout[:, fc, :],
                        start=(fc == 0), stop=(fc == 3))
                sh2b = Pp.tile([128, NCLASS], F32, tag="sh2b", bufs=2)
                nc.vector.tensor_copy(sh2b[:], pH[:, 0:NCLASS])
                nc.sync.dma_start(
                    dblk2.ap()[nc4 * 128:(nc4 + 1) * 128, :]
                    if False else dblk2[nc4 * 128:(nc4 + 1) * 128, :],
                    sh2b[:])
            nc.gpsimd.collective_compute(
                "AllGather", ALU.bypass,
                replica_groups=[list(range(NCORES))],
                ins=[dblk2b[:].opt()], outs=[dgath2b[:].opt()])
            nc.gpsimd.collective_compute(
                "AllGather", ALU.bypass,
                replica_groups=[list(range(NCORES))],
                ins=[dblk2[:].opt()], outs=[dgath2[:].opt()])

            # ======================== layer 2 ========================
            with tc.tile_pool(name="stage2", bufs=1) as P2:
                sh2r = P2.tile([128, JC, NCLASS], F32R, name="sh2r")
                for jc in range(JC):
                    nc.sync.dma_start(
                        sh2r[:, jc, :],
                        dgath2[jc * 128:(jc + 1) * 128, :].bitcast(F32R))
                ident = P2.tile([128, 128], F32, name="ident")
                nc.sync.dma_start(ident[:], id_d.ap())
                sfj2T = P2.tile([128, JC], F32, name="sfj2T")
                nc.sync.dma_start(
                    sfj2T[:].rearrange("p (r jc) -> p r jc", r=8),
                    dgath2b[:].rearrange("r (jc p) -> p r jc", p=128))
                fib2 = P2.tile([128, R], F32, name="fib2")
                nc.gpsimd.partition_broadcast(fib2[:], sfo2[0:1, :])

                # layer-2 attention chunks (batch 4 jc per Exp)
                pagg2 = Pagg.tile([128, 512], F32, tag="agg0", bufs=1)
                pZ2 = Pagg.tile([1, 512], F32, tag="agg1", bufs=1)
                for jb in range(8):
                    raw4 = P2.tile([128, 2048], F32, tag="raw4b", bufs=3)
                    em4 = P2.tile([128, 2048], F32, tag="em4b", bufs=3)
                    P4 = P2.tile([128, 2048], F32R, tag="p4b", bufs=8)
                    for q in range(4):
                        jc = jb * 4 + q
                        sl = slice(q * 512, (q + 1) * 512)
                        mask = P2.tile([128, 512], BF16, tag="maskb", bufs=3)
                        nc.sync.dma_start(
                            mask[:], adj_d.ap()[jc * 128:(jc + 1) * 128, :])
                        nc.vector.scalar_tensor_tensor(
                            raw4[:, sl], mask[:], BIG, fib2[:],
                            op0=ALU.mult, op1=ALU.add)
                        if (jc * 3) % 8 < 3:
                            u = P2.tile([128, 512], F32, tag="ulk2", bufs=3)
                            nc.vector.tensor_scalar_add(
                                u[:], raw4[:, sl], sfj2T[:, jc:jc + 1])
                            nc.vector.scalar_tensor_tensor(
                                em4[:, sl], u[:], ALPHA, u[:],
                                op0=ALU.mult, op1=ALU.max)
                        else:
                            nc.scalar.activation(
                                em4[:, sl], raw4[:, sl], AF.Prelu,
                                bias=sfj2T[:, jc:jc + 1], alpha=alpha[:])
                    nc.scalar.activation(P4[:], em4[:], AF.Exp)
                    for q in range(4):
                        jc = jb * 4 + q
                        sl = slice(q * 512, (q + 1) * 512)
                        nc.tensor.matmul(
                            pagg2[:], sh2r[:, jc, :], P4[:, sl],
                            start=(jc == 0), stop=(jc == JC - 1))
                        nc.tensor.matmul(
                            pZ2[:], onescol[:], P4[:, sl],
                            start=(jc == 0), stop=(jc == JC - 1))

                # normalize, elu, transpose, log_softmax
                sz2 = P2.tile([1, R], F32, name="sz2")
                nc.vector.tensor_copy(sz2[:], pZ2[0:1, :])
                srz2 = P2.tile([1, R], F32, name="srz2")
                nc.vector.reciprocal(srz2[:], sz2[:])
                zb2 = P2.tile([128, R], F32, name="zb2")
                nc.gpsimd.partition_broadcast(zb2[:], srz2[:])
                sv = P2.tile([128, R], F32, name="sv")
                nc.vector.tensor_mul(sv[:], pagg2[:], zb2[:])
                smin = P2.tile([128, R], F32, name="smin")
                nc.vector.tensor_scalar_min(smin[:], sv[:], 0.0)
                sex = P2.tile([128, R], F32, name="sex")
                nc.scalar.activation(sex[:], smin[:], AF.Exp)
                srel = P2.tile([128, R], F32, name="srel")
                nc.scalar.activation(srel[:], sv[:], AF.Relu)
                sres = P2.tile([128, R], F32, name="sres")
                nc.vector.scalar_tensor_tensor(
                    sres[:], sex[:], -1.0, srel[:], op0=ALU.add, op1=ALU.add)

                for it in range(4):
                    ptp = PsS.tile([128, 128], F32, tag="ps_s", bufs=2)
                    nc.tensor.transpose(
                        ptp[:], sres[:, it * 128:(it + 1) * 128], ident[:])
                    st = P2.tile([128, 128], F32, tag="st", bufs=2)
                    nc.vector.tensor_copy(st[:], ptp[:])
                    mx = P2.tile([128, 1], F32, tag="mx", bufs=2)
                    nc.vector.tensor_reduce(
                        mx[:], st[:], axis=mybir.AxisListType.X, op=ALU.max)
                    negmx = P2.tile([128, 1], F32, tag="negmx", bufs=2)
                    nc.vector.tensor_scalar_mul(negmx[:], mx[:], -1.0)
                    sexp = P2.tile([128, 128], F32, tag="sexp", bufs=2)
                    ssum = P2.tile([128, 1], F32, tag="ssum", bufs=2)
                    nc.scalar.activation(
                        sexp[:], st[:], AF.Exp, bias=negmx[:],
                        accum_out=ssum[:])
                    sln = P2.tile([128, 1], F32, tag="sln", bufs=2)
                    nc.scalar.activation(sln[:], ssum[:], AF.Ln)
                    b2 = P2.tile([128, 1], F32, tag="b2", bufs=2)
                    nc.vector.tensor_sub(b2[:], negmx[:], sln[:])
                    sout = P2.tile([128, 128], F32, tag="sout", bufs=2)
                    nc.scalar.activation(sout[:], st[:], AF.Identity, bias=b2[:])
                    nc.sync.dma_start(
                        out_d.ap()[it * 128:(it + 1) * 128, :], sout[:])

    nc.finalize()
    return nc


def _get_nc():
    if "nc" not in _CACHE:
        _CACHE["nc"] = _build_nc()
    return _CACHE["nc"]


def kernel(**inputs):
    x = np.asarray(inputs["x"], dtype=np.float32)
    adj = np.asarray(inputs["adj"])
    W = np.asarray(inputs["W"], dtype=np.float32)
    a = np.asarray(inputs["a"], dtype=np.float32)
    W_out = np.asarray(inputs["W_out"], dtype=np.float32)
    a_out = np.asarray(inputs["a_out"], dtype=np.float32)

    xT = np.ascontiguousarray(x.T)
    Wcat = np.ascontiguousarray(W.transpose(1, 0, 2).reshape(NFEAT, 512))
    WcatT = np.ascontiguousarray(Wcat.T)
    A12 = np.zeros((512, 16), np.float32)
    for hd in range(NHEADS):
        A12[hd * NHID:(hd + 1) * NHID, hd] = a[hd, NHID:]      # a2 -> fj
        A12[hd * NHID:(hd + 1) * NHID, 8 + hd] = a[hd, :NHID]  # a1 -> fi
    WoutT = np.ascontiguousarray(W_out.T)
    AO = np.stack([a_out[:NCLASS], a_out[NCLASS:]], axis=1)
    AO = np.ascontiguousarray(AO, dtype=np.float32)
    ident = np.eye(128, dtype=np.float32)
    adjm1 = adj.astype(np.float32) - 1.0

    in_maps = []
    for c in range(NCORES):
        r0, r1 = c * R, (c + 1) * R
        in_maps.append({
            "xT": xT,
            "xTblk": np.ascontiguousarray(x[r0:r1].T),
            "Wcat": Wcat,
            "WcatT": WcatT,
            "A12": A12,
            "Wout": W_out,
            "WoutT": WoutT,
            "AO": AO,
            "adjm1T": np.ascontiguousarray(adjm1[r0:r1].T).astype(
                ml_dtypes.bfloat16),
            "ident": ident,
        })

    nc = _get_nc()
    trace = bool(os.environ.get("KERNEL_TRACE"))
    res = bass_utils.run_bass_kernel_spmd(
        nc, in_maps, list(range(NCORES)), trace=trace)
    kernel.last_results = res
    out = np.concatenate(
        [res.results[c]["out"] for c in range(NCORES)], axis=0)
    return np.ascontiguousarray(out, dtype=np.float32)
